# revision 8
# baseline (speedup 1.0000x reference)
"""MLA (Multi-head Latent Attention) Bass/Tile kernel for 8 Trainium2 NeuronCores.

Problem: nn_MultiHeadLatentAttention_81707457839331
  B=2, S=2048, HID=2048, NH=16 heads, NOPE=128, ROPE=64, VD=128, QKD=192,
  KVR=512, QR=1536, fp32.

Sharding (single NEFF, SPMD on 8 cores):
  core c -> batch b = c//4, head group g = c%4 (4 heads each).
  Down-projections (q_down, kv_down) are replicated within each 4-core batch
  group (per the sharding hint); q_up/kv_up/attention/o_proj are head-sharded.
  Each core emits a partial o_proj output [S, HID]; the host sums the 4
  partials per batch (unshard step).

On-device layout strategy: everything is kept in [feature, token] ("T")
layouts so no on-device transposes are needed anywhere:
  - matmuls feed each other directly (contraction dim on partitions),
  - RMSNorm per-token scales are folded: gamma into the up-weights (host),
    1/rms(q_lat) into an explicit per-token multiply of qT, 1/rms(c_kv) into
    c_kvT before kv_up,
  - RoPE rotate_half is folded into extra host-rotated weight columns, so
    on-device RoPE is a pure elementwise  x*cos + x_rot*sin,
  - softmax runs unnormalized (scores are bounded, no max subtraction needed);
    probs are renormalized by 1/sum after the attention matmul,
  - causal masking skips strictly-upper score tiles and applies additive
    -1e30 masks (4 precomputed patterns) on diagonal tiles.
All matmuls use float32r (full PE rate at moving dim 512, near-fp32 accuracy).
"""

import numpy as np

import concourse.bass as bass
import concourse.mybir as mybir
import concourse.tile as tile
from concourse import bacc
from concourse.bass import ds, ts
from concourse.bass_utils import run_bass_kernel_spmd

F32 = mybir.dt.float32
F32R = mybir.dt.float32r

B, S, HID, NH = 2, 2048, 2048, 16
NOPE, ROPE, VD = 128, 64, 128
QKD = NOPE + ROPE
KVR, QR = 512, 1536
EPS = 1e-6
SCALE = QKD ** (-0.5)
P = 128

NHC = HID // P            # 16 hidden chunks
NQC = QR // P             # 12 q-latent chunks
NFC = 18                  # total down-proj output chunks (12 qlat + 4 ckv + rope + rot)
NKC = KVR // P            # 4 ckv chunks
NTT = S // 512            # 4 token tiles of 512
NTC = S // P              # 16 token chunks of 128
NDQ = 8                   # q_up output chunks: 4 nope + 2 rope-pairs + 2 rot-pairs
NEG = -1e30


def _r(ap):
    # tiles feeding matmuls are declared float32r directly
    return ap


def _emit(tc):
    nc = tc.nc
    hid_in = nc.dram_tensor("hid", [P, NHC, S], F32R, kind="ExternalInput").ap()
    cos_in = nc.dram_tensor("cos2", [P, S], F32, kind="ExternalInput").ap()
    sin_in = nc.dram_tensor("sin2", [P, S], F32, kind="ExternalInput").ap()
    wd_in = nc.dram_tensor("wd", [NFC, P, NHC, P], F32R, kind="ExternalInput").ap()
    wqup_in = nc.dram_tensor("wqup", [P, NQC, NDQ * P], F32R, kind="ExternalInput").ap()
    wkup_in = nc.dram_tensor("wkup", [P, NKC, 512], F32R, kind="ExternalInput").ap()
    wvup_in = nc.dram_tensor("wvup", [P, NKC, 512], F32R, kind="ExternalInput").ap()
    wo_in = nc.dram_tensor("wo", [P, 4, HID], F32R, kind="ExternalInput").ap()
    out_d = nc.dram_tensor("out", [S, HID], F32, kind="ExternalOutput").ap()

    with (
        tc.tile_pool(name="const", bufs=1) as constp,
        tc.tile_pool(name="dram", bufs=1, space="DRAM") as dramp,
    ):
        ones_f = constp.tile([P, 1], F32)
        nc.vector.memset(ones_f, 1.0)
        ones = constp.tile([P, 1], F32R)
        nc.vector.tensor_copy(ones, ones_f)
        eps_kv = constp.tile([1, 1], F32)
        nc.vector.memset(eps_kv, EPS)
        eps_q = constp.tile([1, 1], F32)
        nc.vector.memset(eps_q, EPS / (SCALE * SCALE))
        # 4 causal additive masks: mask_k[p, x] = 0 if x - p - 128k >= 0 else -1e30
        masks = []
        for k in range(4):
            m = constp.tile([P, 512], F32, name=f"mask{k}")
            nc.gpsimd.memset(m, 0.0)
            nc.gpsimd.affine_select(
                out=m, in_=m, pattern=[[1, 512]],
                compare_op=mybir.AluOpType.is_ge, fill=NEG,
                base=-128 * k, channel_multiplier=-1,
            )
            masks.append(m)
        cos2 = constp.tile([P, S], F32)
        sin2 = constp.tile([P, S], F32)
        nc.sync.dma_start(cos2, cos_in)
        nc.sync.dma_start(sin2, sin_in)

        latT = dramp.tile([NFC, P, S], F32R)     # down-proj outputs [feature, token]
        kt_d = dramp.tile([4, P, S], F32R)       # 4 heads' k_nope.T
        krop_d = dramp.tile([P, S], F32R)        # roped shared key, duplicated halves
        v_d = dramp.tile([NTC, P, 512], F32R)    # V in [token, 4*VD]
        qt_d = dramp.tile([6, P, S], F32R)       # qT: 4 nope + 2 roped pairs

        # ---------------- Phase A: fused down-projection ----------------
        # latT[fc] = (WdT[:, fc].T @ hidT)  for all 18 output chunks
        with (
            tc.tile_pool(name="pa_hid", bufs=1) as ph,
            tc.tile_pool(name="pa_w", bufs=2) as pw,
            tc.tile_pool(name="pa_row", bufs=2) as prow,
            tc.tile_pool(name="pa_ps", bufs=4, space="PSUM") as pps,
        ):
            hid_sb = ph.tile([P, NHC, S], F32R)
            for hc in range(NHC):
                nc.sync.dma_start(hid_sb[:, hc, :], hid_in[:, hc, :])
            for fc in range(NFC):
                w_sb = pw.tile([P, NHC, P], F32R, name="wslice")
                nc.sync.dma_start(w_sb, wd_in[fc])
                row = prow.tile([P, S], F32R, name="arow")
                for tt in range(NTT):
                    ps = pps.tile([P, 512], F32, name="aps")
                    for hc in range(NHC):
                        nc.tensor.matmul(
                            ps, _r(w_sb[:, hc, :]), _r(hid_sb[:, hc, ts(tt, 512)]),
                            start=(hc == 0), stop=(hc == NHC - 1),
                        )
                    nc.vector.tensor_copy(row[:, ts(tt, 512)], ps)
                nc.sync.dma_start(latT[fc], row)

        # ---------------- Phase B: kv norm + rope-k + kv_up ----------------
        with (
            tc.tile_pool(name="pb", bufs=1) as pb,
            tc.tile_pool(name="pb_tmp", bufs=2) as pbt,
            tc.tile_pool(name="pb_row", bufs=2) as pbr,
            tc.tile_pool(name="pb_ps", bufs=3, space="PSUM") as pps2,
            tc.tile_pool(name="pb_pss", bufs=2, space="PSUM") as ppss,
        ):
            kv_sb = pb.tile([P, 6, S], F32R)
            nc.sync.dma_start(kv_sb, latT[12:18].rearrange("c p t -> p c t"))
            rkv_row = pb.tile([1, S], F32)
            for tt in range(NTT):
                ps_s = ppss.tile([1, 512], F32, name="bpss")
                for fc in range(NKC):
                    sq = pbt.tile([P, 512], F32R, name="bsq")
                    nc.scalar.square(sq, kv_sb[:, fc, ts(tt, 512)])
                    nc.tensor.matmul(ps_s, _r(ones), _r(sq),
                                     start=(fc == 0), stop=(fc == NKC - 1))
                sq_s = pbt.tile([1, 512], F32, name="bsqs")
                nc.scalar.activation(sq_s, ps_s, mybir.ActivationFunctionType.Sqrt,
                                     bias=eps_kv, scale=1.0 / KVR)
                nc.vector.reciprocal(rkv_row[:, ts(tt, 512)], sq_s)
            rkv_b = pb.tile([P, S], F32)
            nc.gpsimd.partition_broadcast(rkv_b, rkv_row)
            for fc in range(NKC):
                nc.vector.tensor_mul(kv_sb[:, fc, :], kv_sb[:, fc, :], rkv_b)
            # roped shared key (both 64-halves hold the same data)
            krop_sb = pb.tile([P, S], F32R)
            t1 = pbt.tile([P, S], F32, name="bt1")
            nc.vector.tensor_mul(t1, kv_sb[:, 4, :], cos2)
            nc.vector.tensor_mul(krop_sb, kv_sb[:, 5, :], sin2)
            nc.vector.tensor_add(krop_sb, krop_sb, t1)
            nc.sync.dma_start(krop_d, krop_sb)
            # kv_up
            wk_sb = pb.tile([P, NKC, 512], F32R)
            wv_sb = pb.tile([P, NKC, 512], F32R)
            nc.sync.dma_start(wk_sb, wkup_in)
            nc.sync.dma_start(wv_sb, wvup_in)
            for d in range(4):
                krow = pbr.tile([P, S], F32R, name="krow")
                for tt in range(NTT):
                    ps = pps2.tile([P, 512], F32, name="bps")
                    for fc in range(NKC):
                        nc.tensor.matmul(
                            ps, _r(wk_sb[:, fc, ds(d * P, P)]),
                            _r(kv_sb[:, fc, ts(tt, 512)]),
                            start=(fc == 0), stop=(fc == NKC - 1),
                        )
                    nc.vector.tensor_copy(krow[:, ts(tt, 512)], ps)
                nc.sync.dma_start(kt_d[d], krow)
            for tch in range(NTC):
                ps = pps2.tile([P, 512], F32, name="bpsv")
                for fc in range(NKC):
                    nc.tensor.matmul(
                        ps, _r(kv_sb[:, fc, ds(tch * P, P)]), _r(wv_sb[:, fc, :]),
                        start=(fc == 0), stop=(fc == NKC - 1),
                    )
                vrow = pbr.tile([P, 512], F32R, name="vrow")
                nc.vector.tensor_copy(vrow, ps)
                nc.sync.dma_start(v_d[tch], vrow)

        # ---------------- Phase C: q_up + rope-q + q-norm scale ----------------
        with (
            tc.tile_pool(name="pc_w", bufs=1) as pcw,
            tc.tile_pool(name="pc_slab", bufs=2) as pcs,
            tc.tile_pool(name="pc_q", bufs=2) as pcq,
            tc.tile_pool(name="pc_tmp", bufs=3) as pct,
            tc.tile_pool(name="pc_ps", bufs=4, space="PSUM") as pps3,
            tc.tile_pool(name="pc_pss", bufs=2, space="PSUM") as ppss3,
        ):
            wq_sb = pcw.tile([P, NQC, NDQ * P], F32R)
            nc.sync.dma_start(wq_sb, wqup_in)
            for tt in range(NTT):
                slab = pcs.tile([P, NQC, 512], F32R, name="qslabin")
                nc.sync.dma_start(
                    slab, latT[0:12, :, ts(tt, 512)].rearrange("c p t -> p c t"))
                # per-token 1/rms for q (SCALE folded in)
                ps_s = ppss3.tile([1, 512], F32, name="cpss")
                for fc in range(NQC):
                    sq = pct.tile([P, 512], F32R, name="csq")
                    nc.scalar.square(sq, slab[:, fc, :])
                    nc.tensor.matmul(ps_s, _r(ones), _r(sq),
                                     start=(fc == 0), stop=(fc == NQC - 1))
                rq_s = pct.tile([1, 512], F32, name="crqs")
                nc.scalar.activation(rq_s, ps_s, mybir.ActivationFunctionType.Sqrt,
                                     bias=eps_q,
                                     scale=1.0 / (QR * SCALE * SCALE))
                rq_row = pct.tile([1, 512], F32, name="crqr")
                nc.vector.reciprocal(rq_row, rq_s)
                rq_b = pct.tile([P, 512], F32, name="crqb")
                nc.gpsimd.partition_broadcast(rq_b, rq_row)
                qsl = pcq.tile([P, NDQ, 512], F32R, name="qslabout")
                for d in range(NDQ):
                    ps = pps3.tile([P, 512], F32, name="cps")
                    for fc in range(NQC):
                        nc.tensor.matmul(
                            ps, _r(wq_sb[:, fc, ds(d * P, P)]), _r(slab[:, fc, :]),
                            start=(fc == 0), stop=(fc == NQC - 1),
                        )
                    nc.vector.tensor_copy(qsl[:, d, :], ps)
                # rope combine: chunks 4,5 (rope pairs) with 6,7 (rot pairs)
                for pr in range(2):
                    t1 = pct.tile([P, 512], F32, name="ct1")
                    t2 = pct.tile([P, 512], F32, name="ct2")
                    nc.vector.tensor_mul(t1, qsl[:, 4 + pr, :], cos2[:, ts(tt, 512)])
                    nc.vector.tensor_mul(t2, qsl[:, 6 + pr, :], sin2[:, ts(tt, 512)])
                    nc.vector.tensor_add(qsl[:, 4 + pr, :], t1, t2)
                # apply rq to nope + roped chunks, then store
                for d in range(6):
                    nc.vector.tensor_mul(qsl[:, d, :], qsl[:, d, :], rq_b)
                    nc.sync.dma_start(qt_d[d, :, ts(tt, 512)], qsl[:, d, :])

        # ---------------- Phase D: attention ----------------
        with tc.tile_pool(name="pdf_out", bufs=1) as pdo_sb:
            outHT = pdo_sb.tile([P, 4, S], F32R)
            _attention(tc, pdo_sb, outHT, kt_d, krop_d, v_d, qt_d, ones, masks)
            _o_proj(tc, outHT, wo_in, out_d)


def _attention(tc, pdo_sb, outHT, kt_d, krop_d, v_d, qt_d, ones, masks):
    nc = tc.nc
    with (
        tc.tile_pool(name="pd_kv", bufs=1) as pdkv,
        tc.tile_pool(name="pd_q", bufs=2) as pdq,
        tc.tile_pool(name="pd_e", bufs=4) as pde,
        tc.tile_pool(name="pd_t", bufs=3) as pdt,
        tc.tile_pool(name="pd_psc", bufs=3, space="PSUM") as pdsc,
        tc.tile_pool(name="pd_pso", bufs=2, space="PSUM") as pdo,
        tc.tile_pool(name="pd_pss", bufs=2, space="PSUM") as pdss,
    ):
            kt_sb = pdkv.tile([P, 4, S], F32R)
            nc.sync.dma_start(kt_sb, kt_d.rearrange("c p t -> p c t"))
            krop_sb = pdkv.tile([P, S], F32R)
            nc.sync.dma_start(krop_sb, krop_d)
            v_sb = pdkv.tile([P, NTC, 512], F32R)
            nc.sync.dma_start(v_sb, v_d.rearrange("c p t -> p c t"))

            for h in range(4):
                hb = 64 * (h % 2)
                qn = pdq.tile([P, S], F32R, name="qn")
                nc.sync.dma_start(qn, qt_d[h])
                qp = pdq.tile([P, S], F32R, name="qp")
                nc.sync.dma_start(qp, qt_d[4 + h // 2])
                for i in range(NTT):
                    ps_o = pdo.tile([P, 512], F32, name="pso")
                    ps_s = pdss.tile([1, 512], F32, name="pss")
                    jmax = 4 * i + 3
                    for jc in range(jmax + 1):
                        ps_sc = pdsc.tile([P, 512], F32, name="psc")
                        nc.tensor.matmul(
                            ps_sc, _r(kt_sb[:, h, ds(jc * P, P)]),
                            _r(qn[:, ts(i, 512)]), start=True, stop=False)
                        nc.tensor.matmul(
                            ps_sc, _r(krop_sb[hb:hb + 64, ds(jc * P, P)]),
                            _r(qp[hb:hb + 64, ts(i, 512)]), start=False, stop=True)
                        if jc >= 4 * i:
                            nc.vector.tensor_add(ps_sc, ps_sc, masks[jc - 4 * i])
                        et = pde.tile([P, 512], F32R, name="et")
                        nc.scalar.activation(et, ps_sc,
                                             mybir.ActivationFunctionType.Exp)
                        nc.tensor.matmul(ps_s, _r(ones), _r(et),
                                         start=(jc == 0), stop=(jc == jmax))
                        nc.tensor.matmul(ps_o, _r(v_sb[:, jc, ds(h * P, P)]), _r(et),
                                         start=(jc == 0), stop=(jc == jmax))
                    rs = pdt.tile([1, 512], F32, name="rs")
                    nc.vector.reciprocal(rs, ps_s)
                    rs_b = pdt.tile([P, 512], F32, name="rsb")
                    nc.gpsimd.partition_broadcast(rs_b, rs)
                    nc.vector.tensor_mul(outHT[:, h, ts(i, 512)], ps_o, rs_b)


def _o_proj(tc, outHT, wo_in, out_d):
    nc = tc.nc
    with (
        tc.tile_pool(name="pf_w", bufs=1) as pfw,
        tc.tile_pool(name="pf_row", bufs=2) as pfr,
        tc.tile_pool(name="pf_ps", bufs=4, space="PSUM") as pfp,
    ):
        wo_sb = pfw.tile([P, 4, HID], F32R)
        nc.sync.dma_start(wo_sb, wo_in)
        for tch in range(NTC):
            orow = pfr.tile([P, HID], F32, name="orow")
            for ct in range(4):
                ps = pfp.tile([P, 512], F32, name="fps")
                for hh in range(4):
                    nc.tensor.matmul(
                        ps, _r(outHT[:, hh, ds(tch * P, P)]),
                        _r(wo_sb[:, hh, ts(ct, 512)]),
                        start=(hh == 0), stop=(hh == 3),
                    )
                nc.vector.tensor_copy(orow[:, ts(ct, 512)], ps)
            nc.sync.dma_start(out_d[ds(tch * P, P), :], orow)


_NC_CACHE = None


def _build_nc():
    global _NC_CACHE
    if _NC_CACHE is None:
        nc = bacc.Bacc("TRN2", target_bir_lowering=False, debug=False,
                       num_devices=8)
        with tile.TileContext(nc) as tc:
            _emit(tc)
        nc.compile()
        _NC_CACHE = nc
    return _NC_CACHE


def _shard_inputs(hidden_states, cos, sin, Wq_down, q_gamma, Wq_up,
                  Wkv_down, kv_gamma, Wkv_up, Wo):
    f32 = np.float32
    hid = np.ascontiguousarray(np.asarray(hidden_states, dtype=f32))
    cos = np.asarray(cos, dtype=f32)
    sin = np.asarray(sin, dtype=f32)
    Wqd = np.asarray(Wq_down, dtype=f32)
    Wkd = np.asarray(Wkv_down, dtype=f32)
    qg = np.asarray(q_gamma, dtype=f32)
    kvg = np.asarray(kv_gamma, dtype=f32)
    Wqu = np.asarray(Wq_up, dtype=f32) * qg[None, :]
    Wku = np.asarray(Wkv_up, dtype=f32) * kvg[None, :]
    Wo = np.asarray(Wo, dtype=f32)

    # shared: combined down-proj weight with host-rotated rope columns
    WqdT = Wqd.T                                   # [HID, QR]
    WckvT = Wkd[:KVR].T                            # [HID, KVR]
    krope = Wkd[KVR:].T                            # [HID, 64]
    krot = np.concatenate([-krope[:, 32:], krope[:, :32]], 1)
    WdT = np.concatenate([WqdT, WckvT, krope, krope, krot, krot], 1)  # [HID, 2304]
    wd = np.ascontiguousarray(
        WdT.reshape(NHC, P, NFC, P).transpose(2, 1, 0, 3))  # [18, 128, 16, 128]

    per_batch = []
    for b in range(B):
        h_sw = np.ascontiguousarray(
            hid[b].T.reshape(NHC, P, S).transpose(1, 0, 2))  # [128, 16, 2048]
        cT = cos[b].T                               # [64, S]
        sT = sin[b].T
        cos2 = np.ascontiguousarray(np.concatenate([cT, cT], 0))
        sin2 = np.ascontiguousarray(np.concatenate([sT, sT], 0))
        per_batch.append((h_sw, cos2, sin2))

    per_group = []
    for g in range(4):
        bn, br, brot = [], [], []
        for hl in range(4):
            h = 4 * g + hl
            blk = Wqu[h * QKD:(h + 1) * QKD]       # [192, QR]
            bn.append(blk[:NOPE])
            rr = blk[NOPE:]
            br.append(rr)
            brot.append(np.concatenate([-rr[32:], rr[:32]], 0))
        cols = bn + [np.concatenate([br[0], br[1]], 0),
                     np.concatenate([br[2], br[3]], 0),
                     np.concatenate([brot[0], brot[1]], 0),
                     np.concatenate([brot[2], brot[3]], 0)]
        WquT = np.concatenate(cols, 0).T           # [QR, 1024]
        wqup = np.ascontiguousarray(
            WquT.reshape(NQC, P, NDQ * P).transpose(1, 0, 2))  # [128, 12, 1024]
        kb, vb = [], []
        for hl in range(4):
            h = 4 * g + hl
            blk = Wku[h * (NOPE + VD):(h + 1) * (NOPE + VD)]
            kb.append(blk[:NOPE])
            vb.append(blk[NOPE:])
        WkuT = np.concatenate(kb, 0).T             # [KVR, 512]
        WvuT = np.concatenate(vb, 0).T
        wkup = np.ascontiguousarray(WkuT.reshape(NKC, P, 512).transpose(1, 0, 2))
        wvup = np.ascontiguousarray(WvuT.reshape(NKC, P, 512).transpose(1, 0, 2))
        WoT = Wo[:, g * 512:(g + 1) * 512].T       # [512, HID]
        wo = np.ascontiguousarray(WoT.reshape(4, P, HID).transpose(1, 0, 2))
        per_group.append((wqup, wkup, wvup, wo))

    in_maps = []
    for c in range(8):
        b, g = c // 4, c % 4
        h_sw, cos2, sin2 = per_batch[b]
        wqup, wkup, wvup, wo = per_group[g]
        in_maps.append({
            "hid": h_sw, "cos2": cos2, "sin2": sin2, "wd": wd,
            "wqup": wqup, "wkup": wkup, "wvup": wvup, "wo": wo,
        })
    return in_maps


def kernel(hidden_states, cos, sin, Wq_down, q_gamma, Wq_up,
           Wkv_down, kv_gamma, Wkv_up, Wo, _trace=False):
    nc = _build_nc()
    in_maps = _shard_inputs(hidden_states, cos, sin, Wq_down, q_gamma, Wq_up,
                            Wkv_down, kv_gamma, Wkv_up, Wo)
    res = run_bass_kernel_spmd(nc, in_maps, core_ids=list(range(8)),
                               trace=_trace)
    out = np.zeros((B, S, HID), dtype=np.float32)
    for c in range(8):
        out[c // 4] += res.results[c]["out"]
    if _trace:
        kernel.last_results = res
    return out
